# revision 2
# baseline (speedup 1.0000x reference)
"""LDS forward kernel for Trainium2 (8 NeuronCores, data-parallel over batch).

Math: the reference LDS with diagonal A and d_in == 1 is an exact causal
convolution plus a batch-independent bias:
    out[b,t,o] = sum_{d<=t} K[d,o] x[b,t-d] + bias[t,o]
    K[d,o]     = sum_s B[s] A[s]^d C[s,o]  (+ M[o,0,d-1] for d in 1..KX)
    bias[t,o]  = sum_s h0[s] A[s]^{t+1} C[s,o]
The stacked matrix G = [K; bias] (1024 x 512) is built from 512 decaying
exponentials, so it is numerically low rank: rank 32 reproduces it to
~3e-6.  Host computes (f64) G = U S V^T and splits factors U' (conv
kernels, 512 x 32), P' (bias coefficients, 512 x 32), W' (rank -> output
expansion, 32 x 512).

Device kernel per core (32 batch rows = 8 groups "bg" of 4 interleaved
rows, in 2 quads of 4 bgs):
  stage 1 (conv to rank space): for each (quad, t-block j) a PSUM chain
    over 128-lag chunks dc:  psc[32g+rho, (tau,b)] += Urev[dc]^T mega[bg],
    where mega[k, tau, b] = xpad[b, tau+k] is the 128-shift window built
    by one replicating DMA per bg.  The 4 bgs of a quad occupy the 4
    32-column strips of the PE array (col tiling -> 4 concurrent chains).
  evict: VectorE adds the bias coefficients P and stores c to SBUF bf16.
  stage 2 (expand): out[o, (tau,b)] = W'[:,ob]^T c  with contraction 32;
    the 4 bgs sit in the 4 32-row strips of the array (row tiling -> 4
    concurrent matmuls).  PSUM tiles pair two bgs [128, 1024]; eviction
    (f32->bf16 cast) alternates VectorE / ScalarE.
  store: one DMA per (quad, ob): [128 o-partitions, 16KB contiguous run]
    to a PE-native DRAM layout; host transposes back to [B, T, O].
"""

import numpy as np
import ml_dtypes

BSZ, T, D_IN = 256, 512, 1
S, O, KX = 512, 512, 5
NCORES = 8
BLOC = BSZ // NCORES        # 32 batch rows per core
NBG = BLOC // 4             # 8 groups of 4 batch rows
XPW = 640                   # padded signal width: 127 zeros + 512 + 1 slack
R = 32                      # rank of the factored kernel

_prog_cache = {}
LAST_RESULTS = None         # BassKernelResults of the most recent run


def _build_program():
    import concourse.bacc as bacc
    import concourse.bass as bass
    import concourse.mybir as mybir
    from concourse.tile import TileContext

    f32 = mybir.dt.float32
    bf16 = mybir.dt.bfloat16

    nc = bacc.Bacc("TRN2", target_bir_lowering=False, debug=False)
    # xint[g, i, b] = xpad[g*4 + b, i]  (b-interleaved padded signal)
    xint = nc.dram_tensor("xint", [NBG, XPW, 4], bf16, kind="ExternalInput")
    urev = nc.dram_tensor("urev", [4, 128, R], bf16, kind="ExternalInput")
    wrep = nc.dram_tensor("wrep", [128, 4, 128], bf16, kind="ExternalInput")
    psb = nc.dram_tensor("psb", [128, 4, 128, 4], bf16, kind="ExternalInput")
    # out[quad, ob, o, (j, gp, gi, tau, b)]
    out = nc.dram_tensor("out", [2, 4, 128, 8192], bf16, kind="ExternalOutput")

    with TileContext(nc) as tc:
        with (
            tc.tile_pool(name="consts", bufs=1) as cpool,
            tc.tile_pool(name="mega", bufs=4) as mpool,
            tc.tile_pool(name="csb", bufs=2) as cspool,
            tc.tile_pool(name="osb", bufs=3) as opool,
            tc.tile_pool(name="ps1", bufs=2, space="PSUM") as p1pool,
            tc.tile_pool(name="ps2", bufs=2, space="PSUM") as p2pool,
        ):
            # Const loads on the gpsimd (SWDGE) ring; mega loads + output
            # stores share the sync (SP HWDGE) ring (loads are queued first);
            # evictions alternate VectorE / ScalarE.
            urev_sb = cpool.tile([128, 4, R], bf16, tag="urev")
            nc.gpsimd.dma_start(out=urev_sb[:], in_=urev.ap().rearrange("d k r -> k d r"))
            wrep_sb = cpool.tile([128, 4, 128], bf16, tag="wrep")
            nc.gpsimd.dma_start(out=wrep_sb[:], in_=wrep.ap())
            psb_sb = cpool.tile([128, 4, 128, 4], bf16, tag="psb")
            nc.gpsimd.dma_start(out=psb_sb[:], in_=psb.ap())
            psb_f = psb_sb[:].rearrange("p j t b -> p (j t b)")

            megas = []
            for bg in range(NBG):
                # mega[k, tau, b] = xint[bg, tau + k, b]
                mega = mpool.tile([128, T, 4], bf16, tag="mega")
                src = bass.AP(xint, bg * XPW * 4, [[4, 128], [4, T], [1, 4]])
                nc.sync.dma_start(out=mega[:], in_=src)
                megas.append(mega)

            csbs = [None, None]
            evict_i = 0
            for quad in range(2):
                c_sb = cspool.tile([128, 4 * T], bf16, tag="csb")
                csbs[quad] = c_sb
                for jp in range(2):
                    psc = p1pool.tile([128, 1024], f32, tag="psc")
                    for jh in range(2):
                        j = jp * 2 + jh
                        for dc in range(j + 1):
                            q = j - dc
                            for g in range(4):
                                megaf = megas[quad * 4 + g][:].rearrange("p t b -> p (t b)")
                                nc.tensor.matmul(
                                    psc[32 * g : 32 * g + 32, jh * 512 : jh * 512 + 512],
                                    urev_sb[:, dc, :],
                                    megaf[:, q * 512 : q * 512 + 512],
                                    start=(dc == 0),
                                    stop=(dc == j),
                                    tile_position=(0, 32 * g),
                                )
                    nc.vector.tensor_add(
                        out=c_sb[:, jp * 1024 : jp * 1024 + 1024],
                        in0=psc[:],
                        in1=psb_f[:, jp * 1024 : jp * 1024 + 1024],
                    )
                for ob in range(4):
                    osb = opool.tile([128, 8192], bf16, tag="osb")
                    for j in range(4):
                        for gp in range(2):
                            pso = p2pool.tile([128, 1024], f32, tag="pso")
                            for gi in range(2):
                                g = gp * 2 + gi
                                nc.tensor.matmul(
                                    pso[:, gi * 512 : gi * 512 + 512],
                                    wrep_sb[32 * g : 32 * g + 32, ob, :],
                                    c_sb[32 * g : 32 * g + 32, j * 512 : j * 512 + 512],
                                    start=True,
                                    stop=True,
                                    tile_position=(32 * g, 0),
                                )
                            dst = osb[:, j * 2048 + gp * 1024 : j * 2048 + gp * 1024 + 1024]
                            if evict_i % 2 == 0:
                                nc.vector.tensor_copy(out=dst, in_=pso[:])
                            else:
                                nc.scalar.copy(out=dst, in_=pso[:])
                            evict_i += 1
                    ddst = bass.AP(
                        out, (quad * 4 + ob) * 128 * 8192, [[8192, 128], [1, 8192]]
                    )
                    nc.sync.dma_start(out=ddst, in_=osb[:])
    nc.compile()
    return nc


def _get_program():
    if "p" not in _prog_cache:
        _prog_cache["p"] = _build_program()
    return _prog_cache["p"]


def host_prep(inputs, A, B, C, M, h0):
    """f64 host precompute: rank-R factors of [K; bias] + padded signal."""
    x = inputs[:, :, 0].astype(np.float64)          # [BSZ, T]
    A64 = A.astype(np.float64)
    B64 = B.astype(np.float64)
    C64 = C.astype(np.float64)
    M64 = M.astype(np.float64)
    h64 = h0.astype(np.float64)

    Apow = A64[None, :] ** np.arange(T + 1)[:, None]      # [T+1, S]
    K = (B64[0][None, :] * Apow[:T]) @ C64                # [T, O]
    K[1 : KX + 1, :] += M64[:, 0, :].T                    # AR taps, lags 1..KX
    bias = (h64[None, :] * Apow[1 : T + 1]) @ C64         # [T, O]

    G = np.concatenate([K, bias], axis=0)                 # [2T, O]
    Ug, s, Vt = np.linalg.svd(G, full_matrices=False)
    sc = np.sqrt(s[:R])
    U = Ug[:T, :R] * sc                                   # [T, R] conv kernels
    P = Ug[T:, :R] * sc                                   # [T, R] bias coeffs
    W = Vt[:R] * sc[:, None]                              # [R, O]

    bf = ml_dtypes.bfloat16
    urev = np.ascontiguousarray(U.reshape(4, 128, R)[:, ::-1, :]).astype(bf)
    wrep = np.ascontiguousarray(np.tile(W.reshape(R, 4, 128), (4, 1, 1))).astype(bf)
    # psb[p, j, tau, b] = P[j*128+tau, p % 32]
    psb = np.ascontiguousarray(
        np.tile(P.T.reshape(R, 4, 128)[:, :, :, None], (4, 1, 1, 4))
    ).astype(bf)                                          # [128, 4, 128, 4]

    xpad = np.zeros((BSZ, XPW), np.float32)
    xpad[:, 127 : 127 + T] = x
    xpad = xpad.astype(bf)                                # [BSZ, XPW]
    xint = np.ascontiguousarray(
        xpad.reshape(BSZ // 4, 4, XPW).transpose(0, 2, 1)
    )                                                     # [BSZ//4, XPW, 4]
    return xint, urev, wrep, psb


def kernel(inputs, A, B, C, M, h0):
    global LAST_RESULTS
    from concourse.bass_utils import run_bass_kernel_spmd

    xint, urev, wrep, psb = host_prep(inputs, A, B, C, M, h0)
    nc = _get_program()
    in_maps = [
        {
            "xint": np.ascontiguousarray(xint[c * NBG : (c + 1) * NBG]),
            "urev": urev,
            "wrep": wrep,
            "psb": psb,
        }
        for c in range(NCORES)
    ]
    res = run_bass_kernel_spmd(nc, in_maps, core_ids=list(range(NCORES)))
    LAST_RESULTS = res
    outs = []
    for r in res.results:
        arr = r["out"].reshape(2, 4, 128, 4, 2, 2, 128, 4)
        # [quad, ob, o, j, gp, gi, tau, b] -> [quad, gp, gi, b, j, tau, ob, o]
        arr = arr.transpose(0, 4, 5, 7, 3, 6, 1, 2).reshape(BLOC, T, O)
        outs.append(arr.astype(np.float32))
    return np.concatenate(outs, axis=0)
